# revision 1
# baseline (speedup 1.0000x reference)
"""CrossPSDLoss Trainium2 kernel.

Math (from the reference):
  res = target - pred; both [1024, 16384] f32.
  cross rows i=0..15: row i = concat_b x[b, 1024*i : 1024*(i+1)]  (length 1048576)
  Welch per row: 511 frames of 4096 (stride 2048), periodic-hann*2 window,
  rFFT, power, sum over frames -> S[k].  Loss only uses rows 8..15 and
  frequency bins 21..499 (the (20,500) mask with df=1), and the /T factors
  cancel in the ratio:
     out = (2/480) * sum_{row=8..15} sum_{kb=21..499} S_res[row,kb]/S_tgt[row,kb]

Sharding: one Welch row per NeuronCore (8 rows, 8 cores); each core consumes
only its [1024, 1024] column slice of pred/target.  No collectives; the host
sums the 8 per-core partial scalars.

Per-core pipeline:
  - host pre-casts the slice to bf16 (verified: final rel err ~1e-5)
  - DMA-transpose load -> XT[p, 1024*t + b] = X[b, 128*t + p]  (samples on
    partitions, which the TensorE contraction requires);
    frame_f[k] = XT[p, 1024*t + 2f + q] for k = 1024*q + 128*t + p = 128*j + p
  - res = tgt - pred on DVE (bf16)
  - even/odd fold (win/cos symmetric, sin antisymmetric about k=4096-k):
      u[k] = x[k] + x[4096-k],  v[k] = x[k] - x[4096-k],  k = 0..2047
      Re[n,f] = sum_{k=0..2047} C[k,n] u[k,f] + C[2048,n] x[2048,f]
      Im[n,f] = sum_{k=0..2047} S[k,n] v[k,f]
    built per 128-k-tile j=0..15 as psB = J0 @ B_j (+ row-0 partner
    mini-matmul), U_j = A_j + psB, V_j = A_j - psB on DVE, where
    A_j = y_j, B_j = y_{31-j}, J0 = anti-identity with row 0 zeroed.
    This HALVES the DFT GEMM contraction (16 k-tiles instead of 32).
  - windowed DFT GEMMs vs precomputed folded weights (bins 21..499 only),
    psum [chunk<=120, 511 frames]
  - PSD: Square activation with accum over frames, ratio + reduce on device.
"""

import os
import sys
from contextlib import ExitStack

import numpy as np
import ml_dtypes

for _p in ("/opt/trn_rl_repo", "/root/.axon_site/_ro/trn_rl_repo"):
    if os.path.isdir(_p) and _p not in sys.path:
        sys.path.insert(0, _p)

import concourse.bass as bass
import concourse.mybir as mybir
from concourse import bacc, tile
from concourse.bass_utils import run_bass_kernel_spmd

BF16 = ml_dtypes.bfloat16

NPERSEG = 4096
NSEG = 511
NBINS = 479          # bins 21..499
CHUNKS = [120, 120, 120, 119]   # 479 split into 4 partition chunks
N_CORES = 8
ROW0 = 8             # first Welch row that matters


def _y_ap(xtile, m):
    """AP of y_m[p, f] = frame_f[128*m + p] = XT[p, 1024*t + 2f + q],
    m = 8*q + t, for all 128 partitions and f = 0..510."""
    q, t = divmod(m, 8)
    base = 1024 * t + q
    return xtile[:, base: base + 1021: 2]


def _y0_ap(xtile, m):
    """Row-0 slice of _y_ap(xtile, m); also valid for m == 32 (q=4, t=0),
    whose weight row is zero."""
    q, t = divmod(m, 8)
    base = 1024 * t + q
    return xtile[0:1, base: base + 1021: 2]


def _build_nc() -> bass.Bass:
    # Bacc (not bass.Bass): its compile() runs generate_event_semaphores(),
    # which splits multi-semaphore waits into event-sem chains — TRN2
    # instructions support at most one wait each.
    nc = bacc.Bacc("TRN2", target_bir_lowering=False, debug=False,
                   num_devices=N_CORES)
    dt = mybir.dt

    # x inputs arrive t-major ([t, b, p] with p = column-within-128-block) so
    # every DMA-transpose reads a fully contiguous source (~350 GB/s instead
    # of the ~261 GB/s non-contiguous-mid-dim rate).
    xp_d = nc.dram_tensor("xp", [8, 1024, 128], dt.bfloat16,
                          kind="ExternalInput")
    xt_d = nc.dram_tensor("xt", [8, 1024, 128], dt.bfloat16,
                          kind="ExternalInput")
    wu_d = nc.dram_tensor("wu", [128, 16, NBINS], dt.bfloat16,
                          kind="ExternalInput")
    wv_d = nc.dram_tensor("wv", [128, 16, NBINS], dt.bfloat16,
                          kind="ExternalInput")
    wj0_d = nc.dram_tensor("wj0", [128, 128], dt.bfloat16,
                           kind="ExternalInput")
    w2k_d = nc.dram_tensor("w2k", [1, NBINS], dt.bfloat16,
                           kind="ExternalInput")
    out_d = nc.dram_tensor("out", [1, 1], dt.float32, kind="ExternalOutput")

    with ExitStack() as ctx:
        tc = ctx.enter_context(tile.TileContext(nc))
        xpool = ctx.enter_context(tc.tile_pool(name="x", bufs=1))
        wpool = ctx.enter_context(tc.tile_pool(name="w", bufs=1))
        uvpool = ctx.enter_context(tc.tile_pool(name="uv", bufs=1))
        psb = ctx.enter_context(tc.tile_pool(name="psb", bufs=4, space="PSUM"))
        pspool = ctx.enter_context(tc.tile_pool(name="ps", bufs=3, space="PSUM"))
        ps1 = ctx.enter_context(tc.tile_pool(name="ps1", bufs=1, space="PSUM"))
        scpool = ctx.enter_context(tc.tile_pool(name="sc", bufs=4))
        stat = ctx.enter_context(tc.tile_pool(name="stat", bufs=1))

        wu_sb = wpool.tile([128, 16, NBINS], dt.bfloat16, tag="wu")
        wv_sb = wpool.tile([128, 16, NBINS], dt.bfloat16, tag="wv")
        j0_sb = wpool.tile([128, 128], dt.bfloat16, tag="wj0")
        w2k_sb = wpool.tile([1, NBINS], dt.bfloat16, tag="w2k")
        xt_t = xpool.tile([128, 8192], dt.bfloat16, tag="xt_t")
        xp_t = xpool.tile([128, 8192], dt.bfloat16, tag="xp_t")
        xr_t = xpool.tile([128, 8192], dt.bfloat16, tag="xr_t")

        # DMA order = PE need order: xt tiles + J0 unblock the fold phase of
        # the tgt input first, then the GEMM weights, then xp for res.
        nc.sync.dma_start(j0_sb[:, :], wj0_d[:, :])
        nc.sync.dma_start(w2k_sb[:, :], w2k_d[:, :])
        for t in range(8):
            sl = slice(1024 * t, 1024 * (t + 1))
            nc.sync.dma_start(xt_t[:, sl], xt_d[t], transpose=True)
        nc.sync.dma_start(wu_sb[:, :, :], wu_d[:, :, :])
        nc.sync.dma_start(wv_sb[:, :, :], wv_d[:, :, :])
        for t in range(8):
            sl = slice(1024 * t, 1024 * (t + 1))
            nc.sync.dma_start(xp_t[:, sl], xp_d[t], transpose=True)
        for t in range(8):
            sl = slice(1024 * t, 1024 * (t + 1))
            nc.vector.tensor_sub(xr_t[:, sl], xt_t[:, sl], xp_t[:, sl])

        RATIO = stat.tile([128, 4], dt.float32)
        nc.vector.memset(RATIO[:, :], 0.0)
        ones = stat.tile([128, 1], dt.float32)
        nc.vector.memset(ones[:, :], 1.0)
        # e0: [1, 128] unit row vector; e0.T @ y0 writes y0 into psum row 0
        # and zeros rows 1..127 (full-region group open for the J0 matmul).
        e0 = stat.tile([1, 128], dt.bfloat16)
        nc.vector.memset(e0[:, :], 0.0)
        nc.vector.memset(e0[0:1, 0:1], 1.0)

        # Fold (both inputs first, so the PE's J0 matmuls for input 2 hide
        # the DVE U/V builds of input 1):
        #   psB_j = J0 @ y_{31-j}  (+ row-0 partner y_{32-j}[0]),
        #   U_j = y_j + psB_j, V_j = y_j - psB_j  (bf16, on DVE).
        UV = {}
        for xi, xtile in ((1, xt_t), (0, xr_t)):
            U = []
            V = []
            for j in range(16):
                pb = psb.tile([128, NSEG], dt.float32, tag="psB")
                # Row-0 partner first (e0.T @ y0 — full-region, opens the
                # group), then the J0 matmul closes it: J0's row 0 is
                # all-zero, so it accumulates 0 onto the partner row.
                nc.tensor.matmul(pb[:, :], e0[:, :],
                                 _y0_ap(xtile, 32 - j),
                                 start=True, stop=False)
                nc.tensor.matmul(pb[:, :], j0_sb[:, :], _y_ap(xtile, 31 - j),
                                 start=False, stop=True)
                u = uvpool.tile([128, NSEG], dt.bfloat16, tag=f"U{xi}_{j}")
                v = uvpool.tile([128, NSEG], dt.bfloat16, tag=f"V{xi}_{j}")
                # Bounce psB to SBUF bf16 on ACT so the DVE add/sub run in
                # 2x bf16 mode instead of 1x against fp32 PSUM.
                pbs = scpool.tile([128, NSEG], dt.bfloat16, tag="pbs")
                nc.scalar.copy(pbs[:, :], pb[:, :])
                nc.vector.tensor_add(u[:, :], _y_ap(xtile, j), pbs[:, :])
                nc.vector.tensor_sub(v[:, :], _y_ap(xtile, j), pbs[:, :])
                U.append(u)
                V.append(v)
            UV[xi] = (U, V)

        # E[(xi, trig, c)]: per-bin sum over the 511 frames of out^2 for
        # chunk c of the {cos,sin} DFT of input xi (0=res, 1=tgt).
        E = {}
        for xi, xtile in ((1, xt_t), (0, xr_t)):
            U, V = UV[xi]
            for m in range(8):
                c = m % 4
                trig = m // 4
                rows = CHUNKS[c]
                col0 = 120 * c
                w_sb = wu_sb if trig == 0 else wv_sb
                tiles = U if trig == 0 else V
                ps = pspool.tile([128, NSEG], dt.float32, tag="gemm_ps")
                for j in range(16):
                    nc.tensor.matmul(
                        ps[:rows, :],
                        w_sb[:, j, col0:col0 + rows],
                        tiles[j][:, :],
                        start=(j == 0),
                        stop=(trig == 1 and j == 15),
                    )
                if trig == 0:
                    # k = 2048 singleton (sin weight there is 0)
                    nc.tensor.matmul(
                        ps[:rows, :],
                        w2k_sb[:, col0:col0 + rows],
                        _y0_ap(xtile, 16),
                        start=False, stop=True)
                tmp = scpool.tile([128, NSEG], dt.float32, tag="sq")
                acc = stat.tile([128, 1], dt.float32, tag=f"E{xi}_{m}")
                E[(xi, trig, c)] = acc
                nc.scalar.activation(
                    out=tmp[:rows, :],
                    in_=ps[:rows, :],
                    func=mybir.ActivationFunctionType.Square,
                    accum_out=acc[:rows, :],
                )

        for c in range(4):
            rows = CHUNKS[c]
            sr = stat.tile([128, 1], dt.float32, tag=f"SR{c}")
            st = stat.tile([128, 1], dt.float32, tag=f"ST{c}")
            rec = stat.tile([128, 1], dt.float32, tag=f"REC{c}")
            nc.vector.tensor_add(sr[:rows, :], E[(0, 0, c)][:rows, :],
                                 E[(0, 1, c)][:rows, :])
            nc.vector.tensor_add(st[:rows, :], E[(1, 0, c)][:rows, :],
                                 E[(1, 1, c)][:rows, :])
            nc.vector.reciprocal(rec[:rows, :], st[:rows, :])
            nc.vector.tensor_mul(RATIO[:rows, c:c + 1], sr[:rows, :],
                                 rec[:rows, :])

        tot = ps1.tile([1, 4], dt.float32)
        nc.tensor.matmul(tot[:1, :4], ones[:, :1], RATIO[:, :4],
                         start=True, stop=True)
        scaled = stat.tile([1, 4], dt.float32)
        nc.vector.tensor_scalar_mul(scaled[:1, :], tot[:1, :], 2.0 / 480.0)
        red = stat.tile([1, 1], dt.float32)
        nc.vector.tensor_reduce(red[:1, :1], scaled[:1, :],
                                axis=mybir.AxisListType.X,
                                op=mybir.AluOpType.add)
        nc.sync.dma_start(out_d[:, :], red[:1, :1])

    nc.compile()
    return nc


def _build_w():
    """Folded DFT weights, all bf16:
      wu[p, j, n] = win[k] cos(2 pi k kb_n / 4096), k = 128 j + p  (u weights)
      wv[p, j, n] = win[k] sin(...)                                (v weights)
      wj0 = anti-identity J0[p, 128-p] = 1 for p = 1..127, row 0 zero
      w2k[0, n]  = win[2048] cos(2 pi 2048 kb_n / 4096)
    """
    k = np.arange(NPERSEG, dtype=np.float64)
    win = (0.5 - 0.5 * np.cos(2.0 * np.pi * k / NPERSEG)) * 2.0
    kb = np.arange(21, 21 + NBINS, dtype=np.float64)
    ang = 2.0 * np.pi * np.outer(k, kb) / NPERSEG
    C = win[:, None] * np.cos(ang)
    S = win[:, None] * np.sin(ang)
    wu = np.ascontiguousarray(
        C[:2048].reshape(16, 128, NBINS).transpose(1, 0, 2)).astype(BF16)
    wv = np.ascontiguousarray(
        S[:2048].reshape(16, 128, NBINS).transpose(1, 0, 2)).astype(BF16)
    j0 = np.zeros((128, 128), np.float64)
    for p in range(1, 128):
        j0[p, 128 - p] = 1.0
    w2k = np.ascontiguousarray(C[2048:2049]).astype(BF16)
    return {
        "wu": wu,
        "wv": wv,
        "wj0": j0.astype(BF16),
        "w2k": w2k,
    }


_CACHE: dict = {}


def _get_prog():
    if "nc" not in _CACHE:
        _CACHE["nc"] = _build_nc()
    return _CACHE["nc"]


def _get_w():
    if "w" not in _CACHE:
        _CACHE["w"] = _build_w()
    return _CACHE["w"]


def kernel(pred: np.ndarray, target: np.ndarray, _trace: bool = False):
    nc = _get_prog()
    w = _get_w()
    pred = np.asarray(pred)
    target = np.asarray(target)
    in_maps = []
    for i in range(N_CORES):
        c0 = (ROW0 + i) * 1024
        in_maps.append({
            "xp": np.ascontiguousarray(
                pred[:, c0:c0 + 1024].astype(BF16)
                .reshape(1024, 8, 128).transpose(1, 0, 2)),
            "xt": np.ascontiguousarray(
                target[:, c0:c0 + 1024].astype(BF16)
                .reshape(1024, 8, 128).transpose(1, 0, 2)),
            **w,
        })
    res = run_bass_kernel_spmd(nc, in_maps, list(range(N_CORES)), trace=_trace)
    total = float(sum(float(res.results[i]["out"][0, 0])
                      for i in range(N_CORES)))
    out = np.array(total, dtype=np.float32)
    if _trace:
        return out, res
    return out



# revision 5
# speedup vs baseline: 2.3311x; 2.3311x over previous
"""CrossPSDLoss Trainium2 kernel (fp8 DoubleRow rewrite).

Math (from the reference):
  res = target - pred; both [1024, 16384] f32.
  cross rows i=0..15: row i = concat_b x[b, 1024*i : 1024*(i+1)]  (len 1048576)
  Welch per row: 511 frames of 4096 (stride 2048), periodic-hann*2 window,
  rFFT, power, sum over frames -> S[k].  Loss uses rows 8..15 and bins
  21..499 only; the /T factors cancel in the ratio:
     out = (2/480) * sum_{row=8..15} sum_{kb=21..499} S_res[row,kb]/S_tgt[row,kb]

Sharding: one Welch row per NeuronCore (8 rows, 8 cores); no collectives;
host sums the 8 per-core scalars.

Per-core algorithm (everything fp8 e4m3, tolerance is 2e-2 and fp8
end-to-end sims at ~2e-3):
  - Even/odd fold halves the DFT contraction: for k=1..2047
      u[k,f] = x_f[k] + x_f[4096-k],   v[k,f] = x_f[k] - x_f[4096-k]
      Re[n,f] = sum_k win[k]cos(t n k) u[k,f] + 2(-1)^n x_f[2048]
      Im[n,f] = sum_k win[k]sin(t n k) v[k,f]
    The host builds u/v tensors [128, 16, 512] (k = 128m+p, f packed) for
    target and pred; the k=0 lane is dead (win[0]=0), so the x_f[2048]
    singleton rides in u[0,0,f] with its weight 2(-1)^n written into
    wu[0,0,n] -- no separate singleton matmul.
  - GEMMs are fp8 DoubleRow: each pass contracts TWO 128-k-tiles at 0.5
    cycles/row -- 4x the bf16 rate per unit of contraction.
  - u_res = u_tgt - u_pred on DVE+Pool (hidden under the input DMA).
    The v-side residual is absorbed into the PSUM accumulation instead:
    host ships vpn = -v_pred and the res-sin group runs 8 extra DR passes
    on it.  vpn is DMA'd last, so the post-DMA dependence chain is a few
    106ns PE passes, not a ~5us fp8 DVE subtract.
  - PSD: ACT Square with accum_out per 120-bin chunk; ratio + reduce on
    device; host sums 8 scalars.
"""

import os
import sys
from contextlib import ExitStack

import numpy as np
import ml_dtypes

for _p in ("/opt/trn_rl_repo", "/root/.axon_site/_ro/trn_rl_repo"):
    if os.path.isdir(_p) and _p not in sys.path:
        sys.path.insert(0, _p)

import concourse.bass as bass
import concourse.mybir as mybir
from concourse import bacc, tile
from concourse.bass_utils import run_bass_kernel_spmd

E4 = ml_dtypes.float8_e4m3

NPERSEG = 4096
NSEG = 511
NBINS = 479          # bins 21..499
NCHUNK = 4           # 479 bins in 4 chunks of 120 (chunk 3: 119 real + 1 pad)
ROWS = [120, 120, 120, 119]
N_CORES = 8
ROW0 = 8             # first Welch row that matters


def _build_nc() -> bass.Bass:
    nc = bacc.Bacc("TRN2", target_bir_lowering=False, debug=False,
                   num_devices=N_CORES)
    dt = mybir.dt
    DR = mybir.MatmulPerfMode.DoubleRow

    ut_d = nc.dram_tensor("ut", [128, 16, 512], dt.float8e4, kind="ExternalInput")
    vt_d = nc.dram_tensor("vt", [128, 16, 512], dt.float8e4, kind="ExternalInput")
    up_d = nc.dram_tensor("up", [128, 16, 512], dt.float8e4, kind="ExternalInput")
    vpn_d = nc.dram_tensor("vpn", [128, 16, 512], dt.float8e4, kind="ExternalInput")
    wu_d = nc.dram_tensor("wu", [128, 4, 16, 128], dt.float8e4, kind="ExternalInput")
    wv_d = nc.dram_tensor("wv", [128, 4, 16, 128], dt.float8e4, kind="ExternalInput")
    out_d = nc.dram_tensor("out", [1, 1], dt.float32, kind="ExternalOutput")

    with ExitStack() as ctx:
        tc = ctx.enter_context(tile.TileContext(nc))
        xpool = ctx.enter_context(tc.tile_pool(name="x", bufs=1))
        wpool = ctx.enter_context(tc.tile_pool(name="w", bufs=1))
        pspool = ctx.enter_context(tc.tile_pool(name="ps", bufs=3, space="PSUM"))
        rspool = ctx.enter_context(tc.tile_pool(name="rs", bufs=4, space="PSUM"))
        ps1 = ctx.enter_context(tc.tile_pool(name="ps1", bufs=1, space="PSUM"))
        scpool = ctx.enter_context(tc.tile_pool(name="sc", bufs=4))
        stat = ctx.enter_context(tc.tile_pool(name="stat", bufs=1))

        wu_s = wpool.tile([128, 4, 16, 128], dt.float8e4, tag="wu")
        wv_s = wpool.tile([128, 4, 16, 128], dt.float8e4, tag="wv")
        ut_s = xpool.tile([128, 16, 512], dt.float8e4, tag="ut")
        vt_s = xpool.tile([128, 16, 512], dt.float8e4, tag="vt")
        up_s = xpool.tile([128, 16, 512], dt.float8e4, tag="up")
        vpn_s = xpool.tile([128, 16, 512], dt.float8e4, tag="vpn")
        ur_s = xpool.tile([128, 16, 512], dt.float8e4, tag="ur")

        # DMA order == need order.  vpn last: its only consumers are DR
        # passes (106ns each), so the post-DMA tail is short.
        for c in range(4):
            nc.sync.dma_start(wu_s[:, c], wu_d[:, c])
        for c in range(4):
            nc.sync.dma_start(ut_s[:, 4 * c:4 * c + 4], ut_d[:, 4 * c:4 * c + 4])
        for c in range(4):
            nc.sync.dma_start(up_s[:, 4 * c:4 * c + 4], up_d[:, 4 * c:4 * c + 4])
        for c in range(4):
            nc.sync.dma_start(wv_s[:, c], wv_d[:, c])
        for c in range(4):
            nc.sync.dma_start(vt_s[:, 4 * c:4 * c + 4], vt_d[:, 4 * c:4 * c + 4])
        for c in range(4):
            nc.sync.dma_start(vpn_s[:, 4 * c:4 * c + 4], vpn_d[:, 4 * c:4 * c + 4])

        # Stats: E accumulators as [128, 4] columns (col = bin chunk).
        # Denominator tiles memset to 1 so pad rows give ratio 0/2 = 0.
        e_tc = stat.tile([128, 4], dt.float32, tag="e_tc")
        e_ts = stat.tile([128, 4], dt.float32, tag="e_ts")
        e_rc = stat.tile([128, 4], dt.float32, tag="e_rc")
        e_rs = stat.tile([128, 4], dt.float32, tag="e_rs")
        nc.vector.memset(e_tc[:, :], 1.0)
        nc.vector.memset(e_ts[:, :], 1.0)
        nc.vector.memset(e_rc[:, :], 0.0)
        nc.vector.memset(e_rs[:, :], 0.0)
        ones = stat.tile([128, 1], dt.float32, tag="ones")
        nc.vector.memset(ones[:, :], 1.0)

        # u_res = u_tgt - u_pred (fp8; DVE for m0..7 in two ops, Pool for
        # m8..15 in one big op to amortize its ~1.3us launch overhead).
        nc.vector.tensor_sub(ur_s[:, 0:4], ut_s[:, 0:4], up_s[:, 0:4])
        nc.vector.tensor_sub(ur_s[:, 4:8], ut_s[:, 4:8], up_s[:, 4:8])
        nc.gpsimd.tensor_sub(ur_s[:, 8:16], ut_s[:, 8:16], up_s[:, 8:16])

        def square(ps, acc_tile, c):
            rows = ROWS[c]
            tmp = scpool.tile([128, NSEG], dt.float32, tag="sq")
            nc.scalar.activation(
                out=tmp[:rows, :],
                in_=ps[:rows, :],
                func=mybir.ActivationFunctionType.Square,
                accum_out=acc_tile[:rows, c:c + 1],
            )

        def gemm(w_s, x_tiles, acc_tile, c, pool):
            """One (group, chunk) combo: accumulate DR passes over all
            m-pairs of each tensor in x_tiles, then square+accum."""
            ps = pool.tile([128, NSEG], dt.float32, tag="gps")
            n = len(x_tiles) * 8
            i = 0
            for x_s in x_tiles:
                for j in range(8):
                    nc.tensor.matmul(
                        ps[:120, :],
                        w_s[:, c, 2 * j:2 * j + 2, 0:120],
                        x_s[:, 2 * j:2 * j + 2, 0:511],
                        start=(i == 0), stop=(i == n - 1),
                        perf_mode=DR,
                    )
                    i += 1
            square(ps, acc_tile, c)

        # tgt-cos -> res-cos -> tgt-sin; res-sin absorbs the v-residual
        # (vt passes then vpn passes in one PSUM group).
        for c in range(4):
            gemm(wu_s, [ut_s], e_tc, c, pspool)
        for c in range(4):
            gemm(wu_s, [ur_s], e_rc, c, pspool)
        for c in range(4):
            gemm(wv_s, [vt_s], e_ts, c, pspool)
        for c in range(4):
            gemm(wv_s, [vt_s, vpn_s], e_rs, c, rspool)

        # ratio tail: num = E_rc + E_rs, den = E_tc + E_ts (den pad rows = 2)
        num4 = stat.tile([128, 4], dt.float32, tag="num4")
        den4 = stat.tile([128, 4], dt.float32, tag="den4")
        rec4 = stat.tile([128, 4], dt.float32, tag="rec4")
        rat4 = stat.tile([128, 4], dt.float32, tag="rat4")
        nc.vector.tensor_add(num4[:, :], e_rc[:, :], e_rs[:, :])
        nc.vector.tensor_add(den4[:, :], e_tc[:, :], e_ts[:, :])
        nc.vector.reciprocal(rec4[:, :], den4[:, :])
        nc.vector.tensor_mul(rat4[:, :], num4[:, :], rec4[:, :])

        tot = ps1.tile([1, 4], dt.float32)
        nc.tensor.matmul(tot[:1, :4], ones[:, :1], rat4[:, :4],
                         start=True, stop=True)
        scaled = stat.tile([1, 4], dt.float32)
        nc.vector.tensor_scalar_mul(scaled[:1, :], tot[:1, :], 2.0 / 480.0)
        red = stat.tile([1, 1], dt.float32)
        nc.vector.tensor_reduce(red[:1, :1], scaled[:1, :],
                                axis=mybir.AxisListType.X,
                                op=mybir.AluOpType.add)
        nc.sync.dma_start(out_d[:, :], red[:1, :1])

    nc.compile()
    return nc


def _build_w():
    """DFT weights, fp8 e4m3, bin-chunk-major [128, 4, 16, 120]:
      wu[p, c, m, n] = win[k] cos(2 pi k kb / 4096), k = 128m+p, kb = 21+120c+n
      wv[p, c, m, n] = win[k] sin(...)
      override wu[0, c, 0, n] = 2 cos(pi kb)  (the k=2048 singleton weight;
        the k=0 lane is dead since win[0]=0, and u[0,0,f] carries x_f[2048])
      chunk 3 bin 119 (kb=500) is zero-padded.
    """
    k = np.arange(2048, dtype=np.float64)
    win = 1.0 - np.cos(2.0 * np.pi * k / NPERSEG)          # hann*2, periodic
    kb = np.arange(21, 21 + NBINS, dtype=np.float64)
    ang = 2.0 * np.pi * np.outer(k, kb) / NPERSEG
    C = win[:, None] * np.cos(ang)                          # [2048, 479]
    S = win[:, None] * np.sin(ang)
    C[0, :] = 2.0 * np.cos(np.pi * kb)                      # x[2048] singleton
    S[0, :] = 0.0

    def pack(W):
        # [2048=k, 479=bins] -> [128p, 4c, 16m, 128n]; each chunk's first
        # 120 bin slots are real (chunk 3: 119), the rest zero-padding to
        # keep the DoubleRow weight pair-dim stride 16B-aligned.
        Wp = np.zeros((16, 128, 4, 128), np.float64)
        for c in range(4):
            n = min(120, NBINS - 120 * c)
            Wp[:, :, c, :n] = W[:, 120 * c:120 * c + n].reshape(16, 128, n)
        return np.ascontiguousarray(Wp.transpose(1, 2, 0, 3)).astype(E4)

    return {"wu": pack(C), "wv": pack(S)}


def _fold(row):
    """row: [1048576] f32 -> (U, V) [2048, 511] f32 with the x_f[2048]
    singleton in U[0] and V[0] = 0."""
    R2 = row.reshape(512, 2048)
    Y = R2[:511, :].T                                       # [2048, 511]
    U = np.empty((2048, NSEG), np.float32)
    V = np.empty((2048, NSEG), np.float32)
    YRt = R2[1:512, 1:2048][:, ::-1].T                      # YR[k]=x_f[4096-k]
    U[1:] = Y[1:] + YRt
    V[1:] = Y[1:] - YRt
    U[0] = R2[1:512, 0]                                     # x_f[2048]
    V[0] = 0.0
    return U, V


def _pack_uv(X):
    """[2048, 511] f32 -> [128, 16, 512] fp8 (k = 128m+p, f packed, pad f)."""
    out = np.zeros((128, 16, 512), E4)
    out[:, :, :NSEG] = X.reshape(16, 128, NSEG).transpose(1, 0, 2).astype(E4)
    return out


_CACHE: dict = {}


def _get_prog():
    if "nc" not in _CACHE:
        _CACHE["nc"] = _build_nc()
    return _CACHE["nc"]


def _get_w():
    if "w" not in _CACHE:
        _CACHE["w"] = _build_w()
    return _CACHE["w"]


def kernel(pred: np.ndarray, target: np.ndarray, _trace: bool = False):
    nc = _get_prog()
    w = _get_w()
    pred = np.asarray(pred, np.float32)
    target = np.asarray(target, np.float32)
    in_maps = []
    for i in range(N_CORES):
        c0 = (ROW0 + i) * 1024
        rt = np.ascontiguousarray(target[:, c0:c0 + 1024]).reshape(-1)
        rp = np.ascontiguousarray(pred[:, c0:c0 + 1024]).reshape(-1)
        ut_, vt_ = _fold(rt)
        up_, vp_ = _fold(rp)
        in_maps.append({
            "ut": _pack_uv(ut_),
            "vt": _pack_uv(vt_),
            "up": _pack_uv(up_),
            "vpn": _pack_uv(-vp_),
            **w,
        })
    res = run_bass_kernel_spmd(nc, in_maps, list(range(N_CORES)), trace=_trace)
    total = float(sum(float(res.results[i]["out"][0, 0])
                      for i in range(N_CORES)))
    out = np.array(total, dtype=np.float32)
    if _trace:
        return out, res
    return out


# revision 7
# speedup vs baseline: 3.0057x; 1.2894x over previous
"""CrossPSDLoss Trainium2 kernel (fp8 DoubleRow rewrite).

Math (from the reference):
  res = target - pred; both [1024, 16384] f32.
  cross rows i=0..15: row i = concat_b x[b, 1024*i : 1024*(i+1)]  (len 1048576)
  Welch per row: 511 frames of 4096 (stride 2048), periodic-hann*2 window,
  rFFT, power, sum over frames -> S[k].  Loss uses rows 8..15 and bins
  21..499 only; the /T factors cancel in the ratio:
     out = (2/480) * sum_{row=8..15} sum_{kb=21..499} S_res[row,kb]/S_tgt[row,kb]

Sharding: one Welch row per NeuronCore (8 rows, 8 cores); no collectives;
host sums the 8 per-core scalars.

Per-core algorithm (everything fp8 e4m3, tolerance is 2e-2 and fp8
end-to-end sims at ~2e-3):
  - Even/odd fold halves the DFT contraction: for k=1..2047
      u[k,f] = x_f[k] + x_f[4096-k],   v[k,f] = x_f[k] - x_f[4096-k]
      Re[n,f] = sum_k win[k]cos(t n k) u[k,f] + 2(-1)^n x_f[2048]
      Im[n,f] = sum_k win[k]sin(t n k) v[k,f]
    The host builds u/v tensors [128, 16, 512] (k = 128m+p, f packed) for
    target and pred; the k=0 lane is dead (win[0]=0), so the x_f[2048]
    singleton rides in u[0,0,f] with its weight 2(-1)^n written into
    wu[0,0,n] -- no separate singleton matmul.
  - GEMMs are fp8 DoubleRow: each pass contracts TWO 128-k-tiles at 0.5
    cycles/row -- 4x the bf16 rate per unit of contraction.
  - u_res = u_tgt - u_pred on DVE+Pool (hidden under the input DMA).
    The v-side residual is absorbed into the PSUM accumulation instead:
    host ships vpn = -v_pred and the res-sin group runs 8 extra DR passes
    on it.  vpn is DMA'd last, so the post-DMA dependence chain is a few
    106ns PE passes, not a ~5us fp8 DVE subtract.
  - PSD: ACT Square with accum_out per 120-bin chunk; ratio + reduce on
    device; host sums 8 scalars.
"""

import os
import sys
from contextlib import ExitStack

import numpy as np
import ml_dtypes

for _p in ("/opt/trn_rl_repo", "/root/.axon_site/_ro/trn_rl_repo"):
    if os.path.isdir(_p) and _p not in sys.path:
        sys.path.insert(0, _p)

import concourse.bass as bass
import concourse.mybir as mybir
from concourse import bacc, tile
from concourse.bass_utils import run_bass_kernel_spmd

E4 = ml_dtypes.float8_e4m3

NPERSEG = 4096
NSEG = 511
NBINS = 479          # bins 21..499
NCHUNK = 4           # 479 bins in 4 chunks of 120 (chunk 3: 119 real + 1 pad)
ROWS = [120, 120, 120, 119]
N_CORES = 8
ROW0 = 8             # first Welch row that matters


def _build_nc() -> bass.Bass:
    nc = bacc.Bacc("TRN2", target_bir_lowering=False, debug=False,
                   num_devices=N_CORES)
    dt = mybir.dt
    DR = mybir.MatmulPerfMode.DoubleRow

    ut_d = nc.dram_tensor("ut", [128, 16, 512], dt.float8e4, kind="ExternalInput")
    vt_d = nc.dram_tensor("vt", [128, 16, 512], dt.float8e4, kind="ExternalInput")
    up_d = nc.dram_tensor("up", [128, 16, 512], dt.float8e4, kind="ExternalInput")
    vpn_d = nc.dram_tensor("vpn", [128, 16, 512], dt.float8e4, kind="ExternalInput")
    wu_d = nc.dram_tensor("wu", [128, 4, 16, 128], dt.float8e4, kind="ExternalInput")
    wv_d = nc.dram_tensor("wv", [128, 4, 16, 128], dt.float8e4, kind="ExternalInput")
    out_d = nc.dram_tensor("out", [1, 1], dt.float32, kind="ExternalOutput")

    with ExitStack() as ctx:
        tc = ctx.enter_context(tile.TileContext(nc))
        xpool = ctx.enter_context(tc.tile_pool(name="x", bufs=1))
        wpool = ctx.enter_context(tc.tile_pool(name="w", bufs=1))
        pspool = ctx.enter_context(tc.tile_pool(name="ps", bufs=3, space="PSUM"))
        rspool = ctx.enter_context(tc.tile_pool(name="rs", bufs=4, space="PSUM"))
        ps1 = ctx.enter_context(tc.tile_pool(name="ps1", bufs=1, space="PSUM"))
        scpool = ctx.enter_context(tc.tile_pool(name="sc", bufs=4))
        stat = ctx.enter_context(tc.tile_pool(name="stat", bufs=1))

        wu_s = wpool.tile([128, 4, 16, 128], dt.float8e4, tag="wu")
        wv_s = wpool.tile([128, 4, 16, 128], dt.float8e4, tag="wv")
        ut_s = xpool.tile([128, 16, 512], dt.float8e4, tag="ut")
        vt_s = xpool.tile([128, 16, 512], dt.float8e4, tag="vt")
        up_s = xpool.tile([128, 16, 512], dt.float8e4, tag="up")
        vpn_s = xpool.tile([128, 16, 512], dt.float8e4, tag="vpn")
        ur_s = xpool.tile([128, 16, 512], dt.float8e4, tag="ur")

        # DMA order == need order.  vpn last: its only consumers are DR
        # passes (106ns each), so the post-DMA tail is short.
        for c in range(4):
            nc.sync.dma_start(wu_s[:, c], wu_d[:, c])
        for c in range(4):
            nc.sync.dma_start(ut_s[:, 4 * c:4 * c + 4], ut_d[:, 4 * c:4 * c + 4])
        for c in range(4):
            nc.sync.dma_start(up_s[:, 4 * c:4 * c + 4], up_d[:, 4 * c:4 * c + 4])
        for c in range(4):
            nc.sync.dma_start(wv_s[:, c], wv_d[:, c])
        for c in range(4):
            nc.sync.dma_start(vt_s[:, 4 * c:4 * c + 4], vt_d[:, 4 * c:4 * c + 4])
        for c in range(4):
            nc.sync.dma_start(vpn_s[:, 4 * c:4 * c + 4], vpn_d[:, 4 * c:4 * c + 4])

        # Stats: E accumulators as [128, 4] columns (col = bin chunk).
        # Denominator tiles memset to 1 so pad rows give ratio 0/2 = 0.
        e_tc = stat.tile([128, 4], dt.float32, tag="e_tc")
        e_ts = stat.tile([128, 4], dt.float32, tag="e_ts")
        e_rc = stat.tile([128, 4], dt.float32, tag="e_rc")
        e_rs = stat.tile([128, 4], dt.float32, tag="e_rs")
        nc.vector.memset(e_tc[:, :], 1.0)
        nc.vector.memset(e_ts[:, :], 1.0)
        nc.vector.memset(e_rc[:, :], 0.0)
        nc.vector.memset(e_rs[:, :], 0.0)
        # ones carries the final 2/480 scale so the tail needs no extra op
        ones = stat.tile([128, 1], dt.float32, tag="ones")
        nc.vector.memset(ones[:, :], 2.0 / 480.0)

        # u_res = u_tgt - u_pred (fp8, DVE only -- Pool's gpsimd path costs
        # ~1.7ns/elem + 1.3us launch, slower than DVE's 1.04ns/elem).
        # One op per up DMA chunk so the subs pipeline with the DMA stream.
        for c in range(4):
            nc.vector.tensor_sub(ur_s[:, 4 * c:4 * c + 4],
                                 ut_s[:, 4 * c:4 * c + 4],
                                 up_s[:, 4 * c:4 * c + 4])

        def square(ps, acc_tile, c):
            rows = ROWS[c]
            tmp = scpool.tile([128, NSEG], dt.float32, tag="sq")
            nc.scalar.activation(
                out=tmp[:rows, :],
                in_=ps[:rows, :],
                func=mybir.ActivationFunctionType.Square,
                accum_out=acc_tile[:rows, c:c + 1],
            )

        def gemm(w_s, x_tiles, acc_tile, c, pool):
            """One (group, chunk) combo: accumulate DR passes over all
            m-pairs of each tensor in x_tiles, then square+accum."""
            ps = pool.tile([128, NSEG], dt.float32, tag="gps")
            n = len(x_tiles) * 8
            i = 0
            for x_s in x_tiles:
                for j in range(8):
                    nc.tensor.matmul(
                        ps[:120, :],
                        w_s[:, c, 2 * j:2 * j + 2, 0:120],
                        x_s[:, 2 * j:2 * j + 2, 0:511],
                        start=(i == 0), stop=(i == n - 1),
                        perf_mode=DR,
                    )
                    i += 1
            square(ps, acc_tile, c)

        # tgt-cos -> res-cos -> tgt-sin; res-sin absorbs the v-residual
        # (vt passes then vpn passes in one PSUM group).
        for c in range(4):
            gemm(wu_s, [ut_s], e_tc, c, pspool)
        for c in range(4):
            gemm(wu_s, [ur_s], e_rc, c, pspool)
        for c in range(4):
            gemm(wv_s, [vt_s], e_ts, c, pspool)
        for c in range(4):
            gemm(wv_s, [vt_s, vpn_s], e_rs, c, rspool)

        # ratio tail: num = E_rc + E_rs, den = E_tc + E_ts (den pad rows = 2)
        num4 = stat.tile([128, 4], dt.float32, tag="num4")
        den4 = stat.tile([128, 4], dt.float32, tag="den4")
        rec4 = stat.tile([128, 4], dt.float32, tag="rec4")
        rat4 = stat.tile([128, 4], dt.float32, tag="rat4")
        nc.vector.tensor_add(num4[:, :], e_rc[:, :], e_rs[:, :])
        nc.vector.tensor_add(den4[:, :], e_tc[:, :], e_ts[:, :])
        nc.vector.reciprocal(rec4[:, :], den4[:, :])
        nc.vector.tensor_mul(rat4[:, :], num4[:, :], rec4[:, :])

        tot = ps1.tile([1, 4], dt.float32)
        nc.tensor.matmul(tot[:1, :4], ones[:, :1], rat4[:, :4],
                         start=True, stop=True)
        red = stat.tile([1, 1], dt.float32)
        nc.vector.tensor_reduce(red[:1, :1], tot[:1, :],
                                axis=mybir.AxisListType.X,
                                op=mybir.AluOpType.add)
        nc.sync.dma_start(out_d[:, :], red[:1, :1])

    nc.compile()
    return nc


def _build_w():
    """DFT weights, fp8 e4m3, bin-chunk-major [128, 4, 16, 120]:
      wu[p, c, m, n] = win[k] cos(2 pi k kb / 4096), k = 128m+p, kb = 21+120c+n
      wv[p, c, m, n] = win[k] sin(...)
      override wu[0, c, 0, n] = 2 cos(pi kb)  (the k=2048 singleton weight;
        the k=0 lane is dead since win[0]=0, and u[0,0,f] carries x_f[2048])
      chunk 3 bin 119 (kb=500) is zero-padded.
    """
    k = np.arange(2048, dtype=np.float64)
    win = 1.0 - np.cos(2.0 * np.pi * k / NPERSEG)          # hann*2, periodic
    kb = np.arange(21, 21 + NBINS, dtype=np.float64)
    ang = 2.0 * np.pi * np.outer(k, kb) / NPERSEG
    C = win[:, None] * np.cos(ang)                          # [2048, 479]
    S = win[:, None] * np.sin(ang)
    C[0, :] = 2.0 * np.cos(np.pi * kb)                      # x[2048] singleton
    S[0, :] = 0.0

    def pack(W):
        # [2048=k, 479=bins] -> [128p, 4c, 16m, 128n]; each chunk's first
        # 120 bin slots are real (chunk 3: 119), the rest zero-padding to
        # keep the DoubleRow weight pair-dim stride 16B-aligned.
        Wp = np.zeros((16, 128, 4, 128), np.float64)
        for c in range(4):
            n = min(120, NBINS - 120 * c)
            Wp[:, :, c, :n] = W[:, 120 * c:120 * c + n].reshape(16, 128, n)
        return np.ascontiguousarray(Wp.transpose(1, 2, 0, 3)).astype(E4)

    return {"wu": pack(C), "wv": pack(S)}


def _fold(row):
    """row: [1048576] f32 -> (U, V) [2048, 511] f32 with the x_f[2048]
    singleton in U[0] and V[0] = 0."""
    R2 = row.reshape(512, 2048)
    Y = R2[:511, :].T                                       # [2048, 511]
    U = np.empty((2048, NSEG), np.float32)
    V = np.empty((2048, NSEG), np.float32)
    YRt = R2[1:512, 1:2048][:, ::-1].T                      # YR[k]=x_f[4096-k]
    U[1:] = Y[1:] + YRt
    V[1:] = Y[1:] - YRt
    U[0] = R2[1:512, 0]                                     # x_f[2048]
    V[0] = 0.0
    return U, V


def _pack_uv(X):
    """[2048, 511] f32 -> [128, 16, 512] fp8 (k = 128m+p, f packed, pad f)."""
    out = np.zeros((128, 16, 512), E4)
    out[:, :, :NSEG] = X.reshape(16, 128, NSEG).transpose(1, 0, 2).astype(E4)
    return out


_CACHE: dict = {}


def _get_prog():
    if "nc" not in _CACHE:
        _CACHE["nc"] = _build_nc()
    return _CACHE["nc"]


def _get_w():
    if "w" not in _CACHE:
        _CACHE["w"] = _build_w()
    return _CACHE["w"]


def kernel(pred: np.ndarray, target: np.ndarray, _trace: bool = False):
    nc = _get_prog()
    w = _get_w()
    pred = np.asarray(pred, np.float32)
    target = np.asarray(target, np.float32)
    in_maps = []
    for i in range(N_CORES):
        c0 = (ROW0 + i) * 1024
        rt = np.ascontiguousarray(target[:, c0:c0 + 1024]).reshape(-1)
        rp = np.ascontiguousarray(pred[:, c0:c0 + 1024]).reshape(-1)
        ut_, vt_ = _fold(rt)
        up_, vp_ = _fold(rp)
        in_maps.append({
            "ut": _pack_uv(ut_),
            "vt": _pack_uv(vt_),
            "up": _pack_uv(up_),
            "vpn": _pack_uv(-vp_),
            **w,
        })
    res = run_bass_kernel_spmd(nc, in_maps, list(range(N_CORES)), trace=_trace)
    total = float(sum(float(res.results[i]["out"][0, 0])
                      for i in range(N_CORES)))
    out = np.array(total, dtype=np.float32)
    if _trace:
        return out, res
    return out


# revision 9
# speedup vs baseline: 3.0841x; 1.0261x over previous
"""CrossPSDLoss Trainium2 kernel (fp8 DoubleRow rewrite).

Math (from the reference):
  res = target - pred; both [1024, 16384] f32.
  cross rows i=0..15: row i = concat_b x[b, 1024*i : 1024*(i+1)]  (len 1048576)
  Welch per row: 511 frames of 4096 (stride 2048), periodic-hann*2 window,
  rFFT, power, sum over frames -> S[k].  Loss uses rows 8..15 and bins
  21..499 only; the /T factors cancel in the ratio:
     out = (2/480) * sum_{row=8..15} sum_{kb=21..499} S_res[row,kb]/S_tgt[row,kb]

Sharding: one Welch row per NeuronCore (8 rows, 8 cores); no collectives;
host sums the 8 per-core scalars.

Per-core algorithm (everything fp8 e4m3, tolerance is 2e-2 and fp8
end-to-end sims at ~2e-3):
  - Even/odd fold halves the DFT contraction: for k=1..2047
      u[k,f] = x_f[k] + x_f[4096-k],   v[k,f] = x_f[k] - x_f[4096-k]
      Re[n,f] = sum_k win[k]cos(t n k) u[k,f] + 2(-1)^n x_f[2048]
      Im[n,f] = sum_k win[k]sin(t n k) v[k,f]
    The host builds u/v tensors [128, 16, 512] (k = 128m+p, f packed) for
    target and pred; the k=0 lane is dead (win[0]=0), so the x_f[2048]
    singleton rides in u[0,0,f] with its weight 2(-1)^n written into
    wu[0,0,n] -- no separate singleton matmul.
  - GEMMs are fp8 DoubleRow: each pass contracts TWO 128-k-tiles at 0.5
    cycles/row -- 4x the bf16 rate per unit of contraction.
  - u_res = u_tgt - u_pred on DVE+Pool (hidden under the input DMA).
    The v-side residual is absorbed into the PSUM accumulation instead:
    host ships vpn = -v_pred and the res-sin group runs 8 extra DR passes
    on it.  vpn is DMA'd last, so the post-DMA dependence chain is a few
    106ns PE passes, not a ~5us fp8 DVE subtract.
  - PSD: ACT Square with accum_out per 120-bin chunk; ratio + reduce on
    device; host sums 8 scalars.
"""

import os
import sys
from contextlib import ExitStack

import numpy as np
import ml_dtypes

for _p in ("/opt/trn_rl_repo", "/root/.axon_site/_ro/trn_rl_repo"):
    if os.path.isdir(_p) and _p not in sys.path:
        sys.path.insert(0, _p)

import concourse.bass as bass
import concourse.mybir as mybir
from concourse import bacc, tile
from concourse.bass_utils import run_bass_kernel_spmd

E4 = ml_dtypes.float8_e4m3

NPERSEG = 4096
NSEG = 511
NBINS = 479          # bins 21..499
NCHUNK = 4           # 479 bins in 4 chunks of 120 (chunk 3: 119 real + 1 pad)
ROWS = [120, 120, 120, 119]
N_CORES = 8
ROW0 = 8             # first Welch row that matters


def _build_nc() -> bass.Bass:
    nc = bacc.Bacc("TRN2", target_bir_lowering=False, debug=False,
                   num_devices=N_CORES)
    dt = mybir.dt
    DR = mybir.MatmulPerfMode.DoubleRow

    ut_d = nc.dram_tensor("ut", [128, 16, 512], dt.float8e4, kind="ExternalInput")
    vt_d = nc.dram_tensor("vt", [128, 16, 512], dt.float8e4, kind="ExternalInput")
    up_d = nc.dram_tensor("up", [128, 16, 512], dt.float8e4, kind="ExternalInput")
    vpn_d = nc.dram_tensor("vpn", [128, 16, 512], dt.float8e4, kind="ExternalInput")
    wu_d = nc.dram_tensor("wu", [128, 4, 16, 128], dt.float8e4, kind="ExternalInput")
    wv_d = nc.dram_tensor("wv", [128, 4, 16, 128], dt.float8e4, kind="ExternalInput")
    out_d = nc.dram_tensor("out", [1, 1], dt.float32, kind="ExternalOutput")

    with ExitStack() as ctx:
        tc = ctx.enter_context(tile.TileContext(nc))
        xpool = ctx.enter_context(tc.tile_pool(name="x", bufs=1))
        wpool = ctx.enter_context(tc.tile_pool(name="w", bufs=1))
        pspool = ctx.enter_context(tc.tile_pool(name="ps", bufs=3, space="PSUM"))
        rspool = ctx.enter_context(tc.tile_pool(name="rs", bufs=4, space="PSUM"))
        ps1 = ctx.enter_context(tc.tile_pool(name="ps1", bufs=1, space="PSUM"))
        scpool = ctx.enter_context(tc.tile_pool(name="sc", bufs=4))
        stat = ctx.enter_context(tc.tile_pool(name="stat", bufs=1))

        wu_s = wpool.tile([128, 4, 16, 128], dt.float8e4, tag="wu")
        wv_s = wpool.tile([128, 4, 16, 128], dt.float8e4, tag="wv")
        ut_s = xpool.tile([128, 16, 512], dt.float8e4, tag="ut")
        vt_s = xpool.tile([128, 16, 512], dt.float8e4, tag="vt")
        up_s = xpool.tile([128, 16, 512], dt.float8e4, tag="up")
        vpn_s = xpool.tile([128, 16, 512], dt.float8e4, tag="vpn")
        ur_s = xpool.tile([128, 16, 512], dt.float8e4, tag="ur")

        # DMA order: per-chunk interleave [w_c, tgt_c, pred_c] for the u
        # phase then the v phase, so GEMM passes and the m0-7 subs pipeline
        # chunk-by-chunk with the (strictly serial) DMA stream.
        for c in range(4):
            nc.sync.dma_start(wu_s[:, c], wu_d[:, c])
            nc.sync.dma_start(ut_s[:, 4 * c:4 * c + 4], ut_d[:, 4 * c:4 * c + 4])
            nc.sync.dma_start(up_s[:, 4 * c:4 * c + 4], up_d[:, 4 * c:4 * c + 4])
        for c in range(4):
            nc.sync.dma_start(wv_s[:, c], wv_d[:, c])
            nc.sync.dma_start(vt_s[:, 4 * c:4 * c + 4], vt_d[:, 4 * c:4 * c + 4])
            nc.sync.dma_start(vpn_s[:, 4 * c:4 * c + 4], vpn_d[:, 4 * c:4 * c + 4])

        # Stats: E accumulators as [128, 4] columns (col = bin chunk).
        # Denominator tiles memset to 1 so pad rows give ratio 0/2 = 0.
        e_tc = stat.tile([128, 4], dt.float32, tag="e_tc")
        e_ts = stat.tile([128, 4], dt.float32, tag="e_ts")
        e_rc = stat.tile([128, 4], dt.float32, tag="e_rc")
        e_rs = stat.tile([128, 4], dt.float32, tag="e_rs")
        nc.vector.memset(e_tc[:, :], 1.0)
        nc.vector.memset(e_ts[:, :], 1.0)
        nc.vector.memset(e_rc[:, :], 0.0)
        nc.vector.memset(e_rs[:, :], 0.0)
        # ones carries the final 2/480 scale so the tail needs no extra op
        ones = stat.tile([128, 1], dt.float32, tag="ones")
        nc.vector.memset(ones[:, :], 2.0 / 480.0)

        # Residual hybrid: m0-7 subbed on DVE (arrives early, fully hidden
        # under the remaining DMA); m8-15 absorbed into the res PSUM groups
        # via the host-negated halves of up/vpn.  DVE fp8 runs 1 elem/cycle
        # (no 2x mode for 1-byte dtypes), so a full-tensor sub would trail
        # the DMA stream by ~9us -- half-tensor subs stay off the critical
        # path entirely.
        nc.vector.tensor_sub(ur_s[:, 0:4], ut_s[:, 0:4], up_s[:, 0:4])
        nc.vector.tensor_sub(ur_s[:, 4:8], ut_s[:, 4:8], up_s[:, 4:8])
        vr_s = xpool.tile([128, 8, 512], dt.float8e4, tag="vr")
        nc.vector.tensor_sub(vr_s[:, 0:4], vt_s[:, 0:4], vpn_s[:, 0:4])
        nc.vector.tensor_sub(vr_s[:, 4:8], vt_s[:, 4:8], vpn_s[:, 4:8])

        def square(ps, acc_tile, c):
            rows = ROWS[c]
            tmp = scpool.tile([128, NSEG], dt.float32, tag="sq")
            nc.scalar.activation(
                out=tmp[:rows, :],
                in_=ps[:rows, :],
                func=mybir.ActivationFunctionType.Square,
                accum_out=acc_tile[:rows, c:c + 1],
            )

        def gemm(w_s, passes, acc_tile, c, pool):
            """One (group, chunk) combo: accumulate DR passes (x_tile,
            j-pair list) into one PSUM group, then square+accum."""
            ps = pool.tile([128, NSEG], dt.float32, tag="gps")
            n = sum(len(js) for _, js in passes)
            i = 0
            for x_s, js in passes:
                for j in js:
                    nc.tensor.matmul(
                        ps[:120, :],
                        w_s[:, c, 2 * j:2 * j + 2, 0:120],
                        x_s[:, 2 * j:2 * j + 2, 0:511],
                        start=(i == 0), stop=(i == n - 1),
                        perf_mode=DR,
                    )
                    i += 1
            square(ps, acc_tile, c)

        LO, HI = [0, 1, 2, 3], [4, 5, 6, 7]
        for c in range(4):
            gemm(wu_s, [(ut_s, LO + HI)], e_tc, c, pspool)
        for c in range(4):
            gemm(wu_s, [(ur_s, LO), (ut_s, HI), (up_s, HI)], e_rc, c, pspool)
        for c in range(4):
            gemm(wv_s, [(vt_s, LO + HI)], e_ts, c, pspool)
        for c in range(4):
            gemm(wv_s, [(vr_s, LO), (vt_s, HI), (vpn_s, HI)], e_rs, c, rspool)

        # ratio tail: num = E_rc + E_rs, den = E_tc + E_ts (den pad rows = 2)
        num4 = stat.tile([128, 4], dt.float32, tag="num4")
        den4 = stat.tile([128, 4], dt.float32, tag="den4")
        rec4 = stat.tile([128, 4], dt.float32, tag="rec4")
        rat4 = stat.tile([128, 4], dt.float32, tag="rat4")
        nc.vector.tensor_add(num4[:, :], e_rc[:, :], e_rs[:, :])
        nc.vector.tensor_add(den4[:, :], e_tc[:, :], e_ts[:, :])
        nc.vector.reciprocal(rec4[:, :], den4[:, :])
        nc.vector.tensor_mul(rat4[:, :], num4[:, :], rec4[:, :])

        tot = ps1.tile([1, 4], dt.float32)
        nc.tensor.matmul(tot[:1, :4], ones[:, :1], rat4[:, :4],
                         start=True, stop=True)
        red = stat.tile([1, 1], dt.float32)
        nc.vector.tensor_reduce(red[:1, :1], tot[:1, :],
                                axis=mybir.AxisListType.X,
                                op=mybir.AluOpType.add)
        nc.sync.dma_start(out_d[:, :], red[:1, :1])

    nc.compile()
    return nc


def _build_w():
    """DFT weights, fp8 e4m3, bin-chunk-major [128, 4, 16, 120]:
      wu[p, c, m, n] = win[k] cos(2 pi k kb / 4096), k = 128m+p, kb = 21+120c+n
      wv[p, c, m, n] = win[k] sin(...)
      override wu[0, c, 0, n] = 2 cos(pi kb)  (the k=2048 singleton weight;
        the k=0 lane is dead since win[0]=0, and u[0,0,f] carries x_f[2048])
      chunk 3 bin 119 (kb=500) is zero-padded.
    """
    k = np.arange(2048, dtype=np.float64)
    win = 1.0 - np.cos(2.0 * np.pi * k / NPERSEG)          # hann*2, periodic
    kb = np.arange(21, 21 + NBINS, dtype=np.float64)
    ang = 2.0 * np.pi * np.outer(k, kb) / NPERSEG
    C = win[:, None] * np.cos(ang)                          # [2048, 479]
    S = win[:, None] * np.sin(ang)
    C[0, :] = 2.0 * np.cos(np.pi * kb)                      # x[2048] singleton
    S[0, :] = 0.0

    def pack(W):
        # [2048=k, 479=bins] -> [128p, 4c, 16m, 128n]; each chunk's first
        # 120 bin slots are real (chunk 3: 119), the rest zero-padding to
        # keep the DoubleRow weight pair-dim stride 16B-aligned.
        Wp = np.zeros((16, 128, 4, 128), np.float64)
        for c in range(4):
            n = min(120, NBINS - 120 * c)
            Wp[:, :, c, :n] = W[:, 120 * c:120 * c + n].reshape(16, 128, n)
        return np.ascontiguousarray(Wp.transpose(1, 2, 0, 3)).astype(E4)

    return {"wu": pack(C), "wv": pack(S)}


def _fold(row):
    """row: [1048576] f32 -> (U, V) [2048, 511] f32 with the x_f[2048]
    singleton in U[0] and V[0] = 0."""
    R2 = row.reshape(512, 2048)
    Y = R2[:511, :].T                                       # [2048, 511]
    U = np.empty((2048, NSEG), np.float32)
    V = np.empty((2048, NSEG), np.float32)
    YRt = R2[1:512, 1:2048][:, ::-1].T                      # YR[k]=x_f[4096-k]
    U[1:] = Y[1:] + YRt
    V[1:] = Y[1:] - YRt
    U[0] = R2[1:512, 0]                                     # x_f[2048]
    V[0] = 0.0
    return U, V


def _pack_uv(X):
    """[2048, 511] f32 -> [128, 16, 512] fp8 (k = 128m+p, f packed, pad f)."""
    out = np.zeros((128, 16, 512), E4)
    out[:, :, :NSEG] = X.reshape(16, 128, NSEG).transpose(1, 0, 2).astype(E4)
    return out


_CACHE: dict = {}


def _get_prog():
    if "nc" not in _CACHE:
        _CACHE["nc"] = _build_nc()
    return _CACHE["nc"]


def _get_w():
    if "w" not in _CACHE:
        _CACHE["w"] = _build_w()
    return _CACHE["w"]


def kernel(pred: np.ndarray, target: np.ndarray, _trace: bool = False):
    nc = _get_prog()
    w = _get_w()
    pred = np.asarray(pred, np.float32)
    target = np.asarray(target, np.float32)
    in_maps = []
    for i in range(N_CORES):
        c0 = (ROW0 + i) * 1024
        rt = np.ascontiguousarray(target[:, c0:c0 + 1024]).reshape(-1)
        rp = np.ascontiguousarray(pred[:, c0:c0 + 1024]).reshape(-1)
        ut_, vt_ = _fold(rt)
        up_, vp_ = _fold(rp)
        # pred tensors are mixed-sign: k-tiles m0-7 plain (consumed by the
        # on-device residual subtract), m8-15 negated (absorbed into the
        # res PSUM groups as extra DoubleRow passes)
        up_[1024:] *= -1.0
        vp_[1024:] *= -1.0
        in_maps.append({
            "ut": _pack_uv(ut_),
            "vt": _pack_uv(vt_),
            "up": _pack_uv(up_),
            "vpn": _pack_uv(vp_),
            **w,
        })
    res = run_bass_kernel_spmd(nc, in_maps, list(range(N_CORES)), trace=_trace)
    total = float(sum(float(res.results[i]["out"][0, 0])
                      for i in range(N_CORES)))
    out = np.array(total, dtype=np.float32)
    if _trace:
        return out, res
    return out
